# revision 53
# baseline (speedup 1.0000x reference)
"""Trainium2 Bass kernel for nn_DecoderGRU (attention GRU decoder + vocab head).

Strategy (8 NeuronCores, data-parallel over batch, 8 rows/core):
  - Feature-major layouts; fp16 for every PE operand.
  - Per step the serial chain is: hp (PE) -> energy add (DVE, r-halves) ->
    tanh (ACT) -> scores (PE, b-outer padded psum) -> psum->sbuf copy (DVE)
    -> 4 K=1 transpose matmuls (PE) -> exp (ACT, [128,4] psum) -> context
    as feats-stationary r-contraction matmuls (PE, lo/hi row groups) +
    all-ones matmul for the replicated softmax denominator -> reciprocal
    (DVE) -> normalize fused into ctx psum->sbuf copy -> cgx accumulating
    onto the gate psum (which already holds xg+bias+gh from earlier
    matmuls) -> gate tail (tanh-form sigmoid, exp/tanh stay on one ACT
    table set).
  - fc logits: rows 0-127 interleaved into PE idle windows of steps 16+,
    rows 128-255 at the end.
"""

import threading

import numpy as np

B, R, E, H, V, L = 64, 49, 512, 512, 10000, 33
T = L - 1            # 32 decode steps
NCORES = 8
BL = B // NCORES     # 8 batch rows per core
KT = E // 128        # 4 k-tiles of 128 for E=H=512
M3H = (3 * H) // 128  # 12 m-tiles for gate dim
RH1, RH2 = 25, 24    # r-halves for the energy pipeline
RP = 64              # padded r stride in the score tile
# slot i on device holds batch row PERM[i] of the core's 8 rows
PERM = [0, 2, 4, 6, 1, 3, 5, 7]
NCH = (V + 511) // 512  # 20 vocab chunks of <=512

_BUILD_LOCK = threading.Lock()
_BUILT = {}


def _round_f32r(a):
    """fp32r rounding (drop 13 low mantissa bits, round-to-nearest) on host."""
    v = np.ascontiguousarray(a, dtype=np.float32).view(np.uint32).astype(np.uint64)
    v = (v + 0x1000) & 0xFFFFE000
    return v.astype(np.uint32).view(np.float32)


def _build(has_fcb=True):
    import concourse.mybir as mybir
    import concourse.tile as tile
    from concourse import bacc

    F32 = mybir.dt.float32
    F32R = mybir.dt.float32r
    F16 = mybir.dt.float16
    AF = mybir.ActivationFunctionType
    OP = mybir.AluOpType

    nc = bacc.Bacc("TRN2", target_bir_lowering=False, debug=False,
                   num_devices=NCORES)

    # ---- DRAM I/O (all activation-side tensors already in slot order) ----
    featsT_d = nc.dram_tensor("featsT", [E, R, BL], F32R, kind="ExternalInput")
    featsR_d = nc.dram_tensor("featsR", [128, BL // 2, E], F16,
                              kind="ExternalInput")
    embT_d = nc.dram_tensor("embT", [E, T * BL], F16, kind="ExternalInput")
    attn_We_d = nc.dram_tensor("attn_We", [E, H], F32R, kind="ExternalInput")
    attn_Wh_d = nc.dram_tensor("attn_Wh", [H, H], F16, kind="ExternalInput")
    W_hhT_d = nc.dram_tensor("W_hhT", [H, 2 * H], F16, kind="ExternalInput")
    W_hhn2_d = nc.dram_tensor("W_hhn2", [H, H], F16, kind="ExternalInput")
    W_ihcT_d = nc.dram_tensor("W_ihcT", [E, 3 * H], F16, kind="ExternalInput")
    W_iheT_d = nc.dram_tensor("W_iheT", [E, 3 * H], F16, kind="ExternalInput")
    vw_d = nc.dram_tensor("vw", [H, 1], F16, kind="ExternalInput")
    bsum_d = nc.dram_tensor("bsum", [1, 3 * H], F16, kind="ExternalInput")
    attnb_d = nc.dram_tensor("attnb", [H, 1], F32, kind="ExternalInput")
    fcW_d = nc.dram_tensor("fcW", [H, V], F16, kind="ExternalInput")
    out_d = nc.dram_tensor("out", [T * BL, V], F32, kind="ExternalOutput")

    r3 = lambda ap: ap.rearrange("(kt p) m -> p kt m", p=128)

    with tile.TileContext(nc) as tc:
        with tc.tile_pool(name="persist", bufs=1) as P1:
            # ---- resident tensors (small step-0-critical DMAs first) ----
            attnb = P1.tile([128, KT, 1], F32)
            nc.gpsimd.dma_start(attnb[:], r3(attnb_d.ap()))
            vw = P1.tile([128, KT, 1], F16)
            nc.gpsimd.dma_start(vw[:], r3(vw_d.ap()))
            bsum = P1.tile([1, M3H, 128], F16)
            nc.gpsimd.dma_start(bsum[:], bsum_d.ap().rearrange(
                "a (m p) -> a m p", p=128))
            embT = P1.tile([128, KT, T * BL], F16)
            nc.scalar.dma_start(embT[:], r3(embT_d.ap()))
            W_iheT = P1.tile([128, KT, 3 * H], F16)
            nc.gpsimd.dma_start(W_iheT[:], r3(W_iheT_d.ap()))
            attn_Wh = P1.tile([128, KT, H], F16)
            nc.gpsimd.dma_start(attn_Wh[:], r3(attn_Wh_d.ap()))
            featsR = P1.tile([128, BL // 2, E], F16)
            nc.scalar.dma_start(featsR[:], featsR_d.ap())
            W_hhT = P1.tile([128, KT, 2 * H], F16)
            nc.gpsimd.dma_start(W_hhT[:], r3(W_hhT_d.ap()))
            W_hhn2 = P1.tile([128, KT, H], F16)
            nc.scalar.dma_start(W_hhn2[:], r3(W_hhn2_d.ap()))
            W_ihcT = P1.tile([128, KT, 3 * H], F16)
            nc.gpsimd.dma_start(W_ihcT[:], r3(W_ihcT_d.ap()))
            ones128 = P1.tile([128, 128], F16)
            nc.vector.memset(ones128[:], 1.0)
            one1 = P1.tile([1, 1], F16)
            nc.vector.memset(one1[:], 1.0)
            z16 = P1.tile([128, KT, BL], F16)
            nc.vector.memset(z16[:], 0.0)
            # padded b-outer exp row; pad columns stay zero forever
            ex16 = P1.tile([1, BL, RP], F16)
            nc.vector.memset(ex16[:], 0.0)
            # fp16 hidden-state history (columns t*BL+slot)
            h_all = P1.tile([128, KT, T * BL], F16)
            # energy scratch (written every step)
            en16 = P1.tile([128, KT, R, BL], F16)
            enb = P1.tile([128, KT, R, BL], F16)
            # feat_proj + attn_b, fp16 feature-major
            fpT = P1.tile([128, KT, R, BL], F16)
            # fc weights (DMA issued after precompute DMAs)
            fcW = P1.tile([128, KT, V], F16)
            fcb = None
            if has_fcb:
                fcb_d = nc.dram_tensor("fcb", [1, V], F16,
                                       kind="ExternalInput")
                fcb = P1.tile([128, V], F16)
                nc.scalar.dma_start(fcb[:], fcb_d.ap().to_broadcast((128, V)))

            # ---- precompute: feat_proj = feats @ attn_W[:E] + attn_b ----
            with tc.tile_pool(name="pre", bufs=1) as PP, \
                 tc.tile_pool(name="pre_ps", bufs=2, space="PSUM") as PPS:
                featsT = PP.tile([128, KT, R, BL], F32R)
                nc.sync.dma_start(featsT[:], featsT_d.ap().rearrange(
                    "(kt p) r b -> p kt r b", p=128))
                attn_We = PP.tile([128, KT, H], F32R)
                nc.sync.dma_start(attn_We[:], r3(attn_We_d.ap()))
                for mo in range(KT):
                    ps = PPS.tile([128, R * BL], F32, name="fp_ps")
                    for kt in range(KT):
                        nc.tensor.matmul(
                            ps[:], attn_We[:, kt, mo * 128:(mo + 1) * 128],
                            featsT[:, kt].rearrange("p r b -> p (r b)"),
                            start=(kt == 0), stop=(kt == KT - 1))
                    nc.vector.tensor_scalar(
                        out=fpT[:, mo].rearrange("p r b -> p (r b)"),
                        in0=ps[:], scalar1=attnb[:, mo], scalar2=None,
                        op0=OP.add)


            # fc weights last: small chunks so they never monopolize the
            # DMA engines ahead of the step-critical weight fetches
            for kt in range(KT):
                for vq in range(4):
                    cs = slice(vq * 2500, (vq + 1) * 2500)
                    nc.sync.dma_start(fcW[:, kt, cs],
                                      r3(fcW_d.ap())[:, kt, cs])

            # ---- recurrence ----
            with tc.tile_pool(name="st", bufs=2) as PST, \
                 tc.tile_pool(name="ps_misc", bufs=1, space="PSUM") as PS_M, \
                 tc.tile_pool(name="ps_sc", bufs=1, space="PSUM") as PS_SC, \
                 tc.tile_pool(name="ps_g", bufs=2, space="PSUM") as PS_G, \
                 tc.tile_pool(name="ps_fc", bufs=1, space="PSUM") as PS_FC, \
                 tc.tile_pool(name="fc_sb", bufs=2) as FSB:
                # fc pass-1 schedule: one 2-chunk unit per step from step 16;
                # its psum->sbuf copy runs on DVE early in the NEXT step
                fc1 = {16 + i: (2 * i, min(2 * i + 2, NCH))
                       for i in range((NCH + 1) // 2)}
                fc_pending = None  # (fps, ot, cols, nv) awaiting copy+DMA

                for t in range(T):
                    h_prev = (h_all[:, :, (t - 1) * BL:t * BL] if t > 0
                              else z16[:])

                    # gate psum bank: one long accumulation group
                    # (xg+bias+gh+hn+cgx); only the first matmul starts it
                    # and only the last cgx matmul stops it (one group/bank)
                    gbank = PS_G.tile([128, 512], F32, name="gbank")
                    ghx = gbank[:, 0:M3H * BL].rearrange(
                        "p (m b) -> p m b", b=BL)
                    hn_ps = gbank[:, M3H * BL:(M3H + KT) * BL].rearrange(
                        "p (m b) -> p m b", b=BL)
                    # misc psum bank: hp, ctx, scT, srep; their groups open
                    # and close strictly in program order
                    mbank = PS_M.tile([128, 512], F32, name="mbank")
                    hp = mbank[:, 0:KT * BL].rearrange(
                        "p (k b) -> p k b", b=BL)
                    ctx_ps = mbank[:, KT * BL:2 * KT * BL].rearrange(
                        "p (k b) -> p k b", b=BL)
                    scT = mbank[:, 2 * KT * BL:2 * KT * BL + 4]
                    srep = mbank[:, 2 * KT * BL + 4:2 * KT * BL + 12]
                    emb_t = embT[:, :, t * BL:(t + 1) * BL]
                    for m in range(M3H):
                        for kt in range(KT):
                            nc.tensor.matmul(
                                ghx[:, m], W_iheT[:, kt, m * 128:(m + 1) * 128],
                                emb_t[:, kt], start=(m == 0 and kt == 0),
                                stop=False)
                        nc.tensor.matmul(
                            ghx[:, m], bsum[:, m], ones128[0:1, 0:BL],
                            start=False, stop=False)
                    # hp = attn_Wh @ h  (head of the chain)
                    for mo in range(KT):
                        for kt in range(KT):
                            nc.tensor.matmul(
                                hp[:, mo], attn_Wh[:, kt, mo * 128:(mo + 1) * 128],
                                h_prev[:, kt], start=(kt == 0),
                                stop=(kt == KT - 1))
                    # gh: r,z rows into ghx; n rows at half strength go into
                    # BOTH hn (for r*hn) and ghx (r*hn = hn/2 + th_r*hn/2)
                    for m in range(8):
                        for kt in range(KT):
                            nc.tensor.matmul(
                                ghx[:, m], W_hhT[:, kt, m * 128:(m + 1) * 128],
                                h_prev[:, kt], start=False, stop=False)
                    for m in range(4):
                        for kt in range(KT):
                            nc.tensor.matmul(
                                hn_ps[:, m], W_hhn2[:, kt, m * 128:(m + 1) * 128],
                                h_prev[:, kt], start=False, stop=False)
                        for kt in range(KT):
                            nc.tensor.matmul(
                                ghx[:, 8 + m], W_hhn2[:, kt, m * 128:(m + 1) * 128],
                                h_prev[:, kt], start=False, stop=False)

                    # energy = tanh(fp + hp), pipelined in two r-halves
                    hp16 = PST.tile([128, KT, BL], F16, name="hp16")
                    nc.vector.tensor_copy(hp16[:], hp[:])
                    # separate psum tiles per r-half so the first copy does
                    # not wait on the second half's matmuls (bank-level deps)
                    sc_psA = PS_SC.tile([1, BL, 12], F32, name="sc_psA")
                    sc_psB = PS_SC.tile([1, BL, 18], F32, name="sc_psB")
                    sc_psC = PS_SC.tile([1, BL, 19], F32, name="sc_psC")
                    halves = ((0, 12, sc_psA), (12, 30, sc_psB),
                              (30, R, sc_psC))
                    for (r0, r1, sps) in halves:
                        nr = r1 - r0
                        nc.vector.tensor_tensor(
                            out=en16[:, :, r0:r1], in0=fpT[:, :, r0:r1],
                            in1=hp16[:, :, None, :].to_broadcast(
                                (128, KT, nr, BL)),
                            op=OP.add)
                        nc.scalar.activation(
                            enb[:, :, r0:r1], en16[:, :, r0:r1], AF.Tanh)
                        for kt in range(KT):
                            nc.tensor.matmul(
                                sps[:].rearrange("p b r -> p (b r)"),
                                vw[:, kt],
                                enb[:, kt, r0:r1, :].rearrange("p r b -> p b r"),
                                start=(kt == 0), stop=(kt == KT - 1))
                    # exp straight off the score psum (first half overlaps
                    # the second half's matmuls on the PE)
                    for (r0, r1, sps) in halves:
                        nc.scalar.activation(ex16[:, :, r0:r1], sps[:], AF.Exp)

                    # fc pass-1 copy for the previous step's unit: split into
                    # four low-priority pieces so the scheduler slots them
                    # into idle DVE time without blocking the chain
                    if fc_pending is not None:
                        pfps, pot, pcols, pnv = fc_pending
                        with tc.high_priority(offset=-(10 ** 6)):
                            for qi in range(4):
                                a = qi * 256
                                b = min((qi + 1) * 256, pnv)
                                if a >= b:
                                    continue
                                if has_fcb:
                                    nc.vector.tensor_tensor(
                                        out=pot[:, a:b], in0=pfps[:, a:b],
                                        in1=fcb[:, pcols.start + a:
                                                pcols.start + b], op=OP.add)
                                else:
                                    nc.vector.tensor_copy(
                                        pot[:, a:b], pfps[:, a:b])
                            dma_eng = nc.sync if t % 2 == 0 else nc.scalar
                            dma_eng.dma_start(out_d.ap()[0:128, pcols],
                                              pot[:, :pnv])
                        fc_pending = None

                    # transpose exp(scores) to r-on-partitions: 4 K=1 matmuls
                    exflat = ex16[:].rearrange("p b r -> p (b r)")
                    for c in range(BL // 2):
                        nc.tensor.matmul(
                            scT[:, c:c + 1], exflat[:, c * 128:(c + 1) * 128],
                            one1[:], start=True, stop=True)
                    exTs = PST.tile([128, BL // 2], F16, name="exTs")
                    nc.vector.tensor_copy(exTs[:], scT[:])

                    # replicated softmax denominator (before ctx so the
                    # reciprocal overlaps the ctx matmuls), then context
                    nc.tensor.matmul(srep[:, 0:4], ones128[0:49, :],
                                     exTs[0:49, :], start=True, stop=True)
                    nc.tensor.matmul(srep[:, 4:8], ones128[64:113, :],
                                     exTs[64:113, :], start=True, stop=True)
                    for c in range(BL // 2):
                        for k in range(KT):
                            nc.tensor.matmul(
                                ctx_ps[:, k, 2 * c:2 * c + 1],
                                featsR[0:49, c, k * 128:(k + 1) * 128],
                                exTs[0:49, c:c + 1], start=True, stop=True)
                            nc.tensor.matmul(
                                ctx_ps[:, k, 2 * c + 1:2 * c + 2],
                                featsR[64:113, c, k * 128:(k + 1) * 128],
                                exTs[64:113, c:c + 1], start=True, stop=True)
                    # srep cols: [s0 s2 s4 s6 | s1 s3 s5 s7] (slot parity)
                    rec = PST.tile([128, BL], F32, name="rec")
                    nc.vector.reciprocal(rec[:], srep[:])
                    ctx16 = PST.tile([128, KT, BL], F16, name="ctx16")
                    nc.vector.tensor_tensor(
                        out=ctx16[:].rearrange("p k (j par) -> p k j par",
                                               par=2),
                        in0=ctx_ps[:].rearrange("p k (j par) -> p k j par",
                                                par=2),
                        in1=rec[:].rearrange("p (par j) -> p j par", par=2)[
                            :, None, :, :].to_broadcast((128, KT, 4, 2)),
                        op=OP.mult)

                    # cgx accumulates onto ghx; the very last matmul closes
                    # the bank's accumulation group
                    for m in range(M3H):
                        for kt in range(KT):
                            nc.tensor.matmul(
                                ghx[:, m], W_ihcT[:, kt, m * 128:(m + 1) * 128],
                                ctx16[:, kt], start=False,
                                stop=(m == M3H - 1 and kt == KT - 1))

                    # fc interleave: rows 0-127 during steps 16+; matmuls
                    # here (PE idles during the gate tail), copy next step
                    if t in fc1:
                        c0, c1 = fc1[t]
                        nv = min(512 * c1, V) - 512 * c0
                        cols = slice(c0 * 512, c0 * 512 + nv)
                        fps = PS_FC.tile([128, 1024], F32, name="fc_ps")
                        for ch in range(c0, c1):
                            cnv = min(512, V - ch * 512)
                            for kt in range(KT):
                                nc.tensor.matmul(
                                    fps[:, (ch - c0) * 512:(ch - c0) * 512 + cnv],
                                    h_all[:, kt, 0:128],
                                    fcW[:, kt, ch * 512:ch * 512 + cnv],
                                    start=(kt == 0), stop=(kt == KT - 1))
                        ot = FSB.tile([128, 1024], F32, name="fc_ot")
                        fc_pending = (fps, ot, cols, nv)

                    # gates. r,z = 0.5*(1+tanh(0.5*x)). ghx n-rows already
                    # hold xn + hn/2 + cgx_n, and hn_ps holds hn/2, so
                    # n_pre = ghx_n + tanh(0.5*x_r)*hn/2 with no affine fix.
                    rz_h = PST.tile([128, 8, BL], F32, name="rz_h")
                    nc.scalar.activation(rz_h[:], ghx[:, 0:8], AF.Tanh,
                                         scale=0.5)
                    npre = PST.tile([128, KT, BL], F32, name="npre")
                    t_r = PST.tile([128, KT, BL], F32, name="t_r")
                    nc.vector.tensor_tensor(
                        out=t_r[:], in0=rz_h[:, 0:4], in1=hn_ps[:], op=OP.mult)
                    nc.vector.tensor_tensor(
                        out=npre[:], in0=ghx[:, 8:12], in1=t_r[:], op=OP.add)
                    # z and 1-z, and z*h ahead of the final tanh
                    z_g = PST.tile([128, KT, BL], F16, name="z_g")
                    zc_g = PST.tile([128, KT, BL], F16, name="zc_g")
                    nc.vector.tensor_scalar(
                        out=z_g[:], in0=rz_h[:, 4:8], scalar1=0.5,
                        scalar2=0.5, op0=OP.mult, op1=OP.add)
                    nc.vector.tensor_scalar(
                        out=zc_g[:], in0=rz_h[:, 4:8], scalar1=-0.5,
                        scalar2=0.5, op0=OP.mult, op1=OP.add)
                    q_g = PST.tile([128, KT, BL], F16, name="q_g")
                    nc.vector.tensor_tensor(
                        out=q_g[:], in0=z_g[:], in1=h_prev, op=OP.mult)
                    n_t = PST.tile([128, KT, BL], F16, name="n_t")
                    nc.scalar.activation(n_t[:], npre[:], AF.Tanh)
                    # h_new = z*h + (1-z)*n, written straight into h_all
                    w_g = PST.tile([128, KT, BL], F16, name="w_g")
                    nc.vector.tensor_tensor(
                        out=w_g[:], in0=zc_g[:], in1=n_t[:], op=OP.mult)
                    nc.vector.tensor_tensor(
                        out=h_all[:, :, t * BL:(t + 1) * BL], in0=q_g[:],
                        in1=w_g[:], op=OP.add)

            # ---- fc pass 2: rows 128-255 ----
            with tc.tile_pool(name="fc2_ps", bufs=4, space="PSUM") as FPS2, \
                 tc.tile_pool(name="fc2_sb", bufs=4) as FSB2:
                for ch in range(NCH):
                    nv = min(512, V - ch * 512)
                    cols = slice(ch * 512, ch * 512 + nv)
                    ps = FPS2.tile([128, 512], F32, name="fc2_ps")
                    for kt in range(KT):
                        nc.tensor.matmul(
                            ps[:, :nv], h_all[:, kt, 128:256],
                            fcW[:, kt, cols], start=(kt == 0),
                            stop=(kt == KT - 1))
                    ot = FSB2.tile([128, 512], F32, name="fc2_ot")
                    if has_fcb:
                        nc.vector.tensor_tensor(
                            out=ot[:, :nv], in0=ps[:, :nv],
                            in1=fcb[:, cols], op=OP.add)
                    elif ch % 2 == 0:
                        nc.vector.tensor_copy(ot[:, :nv], ps[:, :nv])
                    else:
                        nc.scalar.copy(ot[:, :nv], ps[:, :nv])
                    dma_eng = nc.sync if ch % 2 == 0 else nc.scalar
                    dma_eng.dma_start(out_d.ap()[128:256, cols], ot[:, :nv])

    nc.compile()
    return nc


def _get_built(has_fcb=True):
    with _BUILD_LOCK:
        if has_fcb not in _BUILT:
            _BUILT[has_fcb] = _build(has_fcb)
    return _BUILT[has_fcb]


def kernel(features, captions, embed_table, attn_W, attn_b, v_w,
           W_ih, W_hh, b_ih, b_hh, fc_W, fc_b):
    from concourse.bass_utils import run_bass_kernel_spmd

    features = np.asarray(features, dtype=np.float32)
    captions = np.asarray(captions)
    embed_table = np.asarray(embed_table, dtype=np.float32)
    attn_W = np.asarray(attn_W, dtype=np.float32)
    attn_b = np.asarray(attn_b, dtype=np.float32)
    v_w = np.asarray(v_w, dtype=np.float32)
    W_ih = np.asarray(W_ih, dtype=np.float32)
    W_hh = np.asarray(W_hh, dtype=np.float32)
    b_ih = np.asarray(b_ih, dtype=np.float32)
    b_hh = np.asarray(b_hh, dtype=np.float32)
    fc_W = np.asarray(fc_W, dtype=np.float32)
    fc_b = np.asarray(fc_b, dtype=np.float32)

    has_fcb = bool(np.any(fc_b))
    nc = _get_built(has_fcb)

    f16 = np.float16
    shared = {
        "attn_We": _round_f32r(attn_W[:E]),
        "attn_Wh": attn_W[E:].astype(f16),
        "W_hhT": np.ascontiguousarray(W_hh[:2 * H].T).astype(f16),
        "W_hhn2": np.ascontiguousarray(0.5 * W_hh[2 * H:].T).astype(f16),
        "W_ihcT": np.ascontiguousarray(W_ih[:, E:].T).astype(f16),
        "W_iheT": np.ascontiguousarray(W_ih[:, :E].T).astype(f16),
        "vw": v_w[:, None].astype(f16),
        "bsum": (b_ih + b_hh)[None, :].astype(f16),
        "attnb": np.ascontiguousarray(attn_b[:, None]),
        "fcW": fc_W.astype(f16),
    }
    if has_fcb:
        shared["fcb"] = fc_b[None, :].astype(f16)
    emb = embed_table[captions[:, :T].astype(np.int64)]  # [B, T, E]
    perm = np.array(PERM)
    in_maps = []
    for c in range(NCORES):
        rows = c * BL + perm             # batch rows in slot order
        fr = features[rows]              # [BL, R, E] slot-ordered
        featsR = np.zeros((128, BL // 2, E), dtype=np.float32)
        for cc in range(BL // 2):
            featsR[0:49, cc] = fr[2 * cc]
            featsR[64:113, cc] = fr[2 * cc + 1]
        m = dict(shared)
        m["featsT"] = _round_f32r(fr.transpose(2, 1, 0))
        m["featsR"] = featsR.astype(f16)
        m["embT"] = emb[rows].transpose(2, 1, 0).reshape(E, T * BL).astype(f16)
        in_maps.append(m)

    res = run_bass_kernel_spmd(nc, in_maps, core_ids=list(range(NCORES)))

    out = np.empty((B, T, V), dtype=np.float32)
    for c in range(NCORES):
        # per-core output rows are t*BL + slot; slot i is batch PERM[i]
        r = res.results[c]["out"].reshape(T, BL, V)
        out[c * BL + perm] = r.transpose(1, 0, 2)
    return out
